# revision 23
# baseline (speedup 1.0000x reference)
"""Trainium2 Bass kernel for nn_BinaryResNetBlock (XNOR-style binary ResNet block).

Math (per reference):
  a1 = sign(x);  y1 = conv3x3(a1, s1*sign(w1));  inner = sign(BN_train(y1))
  y2 = conv3x3(inner, s2*sign(w2));  out = sign(BN_train(y2) + x)

Key facts exploited:
  - conv inputs are exactly {-1,0,+1} and sign(w) in {-1,0,+1}: the conv result
    is (per-channel scale s) * (exact small integer n), computed exactly with
    fp8 DoubleRow matmuls accumulating in fp32 PSUM; n stored fp16.
  - BN(y)*gamma+beta = A*n + B with per-channel A, B from global batch stats
    => cross-core AllReduce of tiny per-channel sums only.
  - sign(BN1(y1)) with beta1==0 reduces to sign(gamma1*(n - mean_n)): no var
    needed for stage 1.
  - conv is linear, so sum_batch,space(n1) = sum_taps w1 . S where S[k,tap] is
    a windowed sum of a1 = sign(x) (total minus boundary strips). S is known at
    the END OF STAGE 0, so the stats-1 AllReduce runs DURING conv1's matmuls
    and the BN1 affine is ready right when conv1 finishes (mean recovered by
    36 tiny f16 matmuls against sign(w1)).

Layout note: a_buf is [128, img, j, IMG_STR] img-major so a matmul rhs
[128, (j), FREE] spans a byte range confined to ONE image -- keeps the Tile
framework's dependency ranges per-image, which is what lets stage0/conv1 and
inner/conv2 pipeline image-by-image, and lets the NEXT rep's stage0/conv1
overlap THIS rep's final drain.

Rep-level software pipeline: the final pass for o-chunk 1 (x load + fused
affine_then_add + sign + store) is NOT emitted in its own rep's tail; the
work is carried and emitted interleaved into the NEXT rep's stage0/conv1
loop (7 tiles per image iteration, stores lagging 3 tiles).  Its stats-2
AllReduce thus runs while the next rep's conv1 matmuls keep the PE busy.
The last rep flushes the carry at the end.

Queue assignment (steady state, per image iteration of phase A, ~12.2us of
PE work): SP: 4 x-tile half loads (9.7); Pool: carried x loads + stores
(9.7) + AR1 share; DVE: 11 conv1 evictions + strips + carried fused-affine
(12.1); ACT: 4 stage0 signs + 3 conv1 evictions + carried signs (12.1).
"""

import numpy as np
import ml_dtypes

# ---- problem constants (hardcoded; kernel.py must be self-contained) ----
NCORES = 8
NTOT = 64          # total batch
NIMG = NTOT // NCORES
CH = 256           # in/out channels
H = W = 56
HW = H * W         # 3136
HHW = HW // 2      # half image (28 rows)
WPAD = 58          # padded width (one col pad each side)
IMG_STR = 3366     # padded image stride: 58*58 (+2 tail for rg6 matmul overread)
RH = 8             # output rows per tile
RG = H // RH       # 7 row groups
FREE = RH * WPAD   # 464 matmul free size (includes junk columns)
VW = RH * W        # 448 valid elements per tile
NTILE = NIMG * RG  # 56 tiles per (oc)
COUNT_TOT = NTOT * HW  # 200704 elements per channel for BN stats
EPS = 1e-5

_CACHE = {}
LAST_RESULT = None  # BassKernelResults of the most recent run (for test harness)


def _build(fast1: bool, dbg: bool = False, reps: int = 1, sync_level: int = 1):
    import os
    import concourse.bacc as bacc
    import concourse.mybir as mybir
    import concourse.tile as tile
    from itertools import product

    F8 = mybir.dt.float8e4
    F16 = mybir.dt.float16
    F32 = mybir.dt.float32
    AF = mybir.ActivationFunctionType
    ALU = mybir.AluOpType
    DR = mybir.MatmulPerfMode.DoubleRow
    no_ar = os.environ.get("KERNEL_NO_AR", "0") == "1"

    nc = bacc.Bacc("TRN2", target_bir_lowering=False, debug=False,
                   enable_asserts=True, num_devices=NCORES)

    x_d = nc.dram_tensor("x", [NIMG, CH, H, W], F32, kind="ExternalInput").ap()
    wq_d = nc.dram_tensor("wq", [128, 2, 3, 3, 2, 2, 128], F8,
                          kind="ExternalInput").ap()
    wm_d = nc.dram_tensor("wm", [128, 3, 3, 2, 2, 128], F16,
                          kind="ExternalInput").ap()
    cf_d = nc.dram_tensor("cf", [128, 2, 6], F32, kind="ExternalInput").ap()
    out_d = nc.dram_tensor("out", [NIMG, CH, H, W], F32,
                           kind="ExternalOutput").ap()

    with tile.TileContext(nc) as tc:
        with tc.tile_pool(name="big", bufs=1) as big, \
             tc.tile_pool(name="small", bufs=1) as small, \
             tc.tile_pool(name="xst", bufs=2) as xst, \
             tc.tile_pool(name="xr0", bufs=8) as xr0p, \
             tc.tile_pool(name="xr1", bufs=4) as xr1p, \
             tc.tile_pool(name="ps", bufs=7, space="PSUM") as pspool, \
             tc.tile_pool(name="pm", bufs=1, space="PSUM") as pmpool, \
             tc.tile_pool(name="dr", bufs=1, space="DRAM") as dr:

            # persistent buffers
            a_buf = big.tile([128, NIMG, 2, IMG_STR], F8)    # padded +-1 acts
            m_buf = big.tile([128, 2, NIMG, HW], F16)        # conv ints (m1 then m2)
            wsb = small.tile([128, 2, 3, 3, 2, 2, 128], F8)  # signed weights (fp8)
            w16 = small.tile([128, 3, 3, 2, 2, 128], F16)    # signed w1 (fp16)
            cf = small.tile([128, 2, 6], F32)                # s1,g1,b1,s2,g2,b2
            # windowed-sum scratch: T_h0,T_h1,R0,R55,C0h0,C0h1,C55h0,C55h1
            sacc = small.tile([128, 2, NIMG, 8], F32)
            st1 = small.tile([128, 2, NTILE, 6], F32) if not fast1 else None
            st2 = small.tile([128, 2, NTILE, 6], F32)        # bn_stats (conv2)
            ab1 = small.tile([128, 2, 2], F32)               # A1, B1
            ab2 = small.tile([128, 2, 2], F32)               # A2, B2
            acc1 = small.tile([128, 2, NTILE], F32) if not fast1 else None

            nc.sync.dma_start(wsb[:], wq_d[:])
            nc.sync.dma_start(w16[:], wm_d[:])
            nc.sync.dma_start(cf[:], cf_d[:])

            # zero the padding cells of a_buf (stay zero for both conv inputs)
            nc.gpsimd.memset(a_buf[:, :, :, 0:WPAD], 0.0)              # row -1
            nc.gpsimd.memset(a_buf[:, :, :, 57 * WPAD:IMG_STR], 0.0)   # row 56 + tail
            colpad = a_buf[:, :, :, 57:57 + 57 * WPAD].rearrange(
                "p i j (r t) -> p i j r t", t=WPAD)[:, :, :, :, 0:2]   # col pads
            nc.gpsimd.memset(colpad, 0.0)

            def interior(j, img, r0, nrows):
                """[128, nrows, 56] view of the valid cells of a_buf."""
                return a_buf[:, img, j, 0:3364].rearrange(
                    "p (r w) -> p r w", w=WPAD)[:, 1 + r0:1 + r0 + nrows, 1:57]

            # general-path affine (DVE/ACT engines, one blob)
            def make_affine_old(gsb, ab, oc, si, gi, bi):
                gm = small.tile([128, 1], F32, name=f"gm{si}_{oc}")
                gv = small.tile([128, 1], F32, name=f"gv{si}_{oc}")
                gq = small.tile([128, 1], F32, name=f"gq{si}_{oc}")
                t0 = small.tile([128, 1], F32, name=f"t0{si}_{oc}")
                t1 = small.tile([128, 1], F32, name=f"t1{si}_{oc}")
                sc = cf[:, oc, si:si + 1]
                gc = cf[:, oc, gi:gi + 1]
                bc = cf[:, oc, bi:bi + 1]
                nc.vector.tensor_scalar_mul(gm[:], gsb[:, 0:1], 1.0 / NCORES)
                nc.vector.tensor_scalar_mul(gv[:], gsb[:, 1:2], 1.0 / NCORES)
                nc.vector.tensor_scalar_mul(gq[:], gsb[:, 2:3], 1.0 / NCORES)
                nc.vector.tensor_tensor(t0[:], gm[:], gm[:], ALU.mult)
                nc.vector.tensor_sub(t0[:], gq[:], t0[:])
                nc.vector.tensor_add(t0[:], gv[:], t0[:])
                nc.vector.tensor_tensor(t1[:], sc, sc, ALU.mult)
                nc.vector.tensor_tensor(t0[:], t0[:], t1[:], ALU.mult)
                nc.vector.tensor_scalar_add(t0[:], t0[:], EPS)
                nc.scalar.sqrt(t0[:], t0[:])
                nc.vector.reciprocal(t0[:], t0[:])
                nc.vector.tensor_tensor(t1[:], gc, sc, ALU.mult)
                nc.vector.tensor_tensor(t1[:], t1[:], t0[:], ALU.mult)
                nc.vector.tensor_copy(ab[:, oc, 0:1], t1[:])
                nc.vector.tensor_tensor(t1[:], gm[:], t1[:], ALU.mult)
                nc.vector.tensor_sub(ab[:, oc, 1:2], bc, t1[:])

            # fast-path conv2 affine: Pool pre-ops, ACT sqrt, DVE recip, Pool
            # A/B.  Emission position chosen by caller so no queue stalls on
            # not-yet-available inputs while it still has conv work.
            def make_affine2(gsb, ab, oc, si, gi, bi):
                gm = small.tile([128, 1], F32, name=f"f_gm_{oc}")
                gv = small.tile([128, 1], F32, name=f"f_gv_{oc}")
                gq = small.tile([128, 1], F32, name=f"f_gq_{oc}")
                t0 = small.tile([128, 1], F32, name=f"f_t0_{oc}")
                t1 = small.tile([128, 1], F32, name=f"f_t1_{oc}")
                sc = cf[:, oc, si:si + 1]
                gc = cf[:, oc, gi:gi + 1]
                bc = cf[:, oc, bi:bi + 1]
                nc.gpsimd.tensor_scalar_mul(gm[:], gsb[:, 0:1], 1.0 / NCORES)
                nc.gpsimd.tensor_scalar_mul(gv[:], gsb[:, 1:2], 1.0 / NCORES)
                nc.gpsimd.tensor_scalar_mul(gq[:], gsb[:, 2:3], 1.0 / NCORES)
                nc.gpsimd.tensor_tensor(t0[:], gm[:], gm[:], ALU.mult)
                nc.gpsimd.tensor_sub(t0[:], gq[:], t0[:])
                nc.gpsimd.tensor_add(t0[:], gv[:], t0[:])
                nc.gpsimd.tensor_tensor(t1[:], sc, sc, ALU.mult)
                nc.gpsimd.tensor_tensor(t0[:], t0[:], t1[:], ALU.mult)
                nc.gpsimd.tensor_scalar_add(t0[:], t0[:], EPS)
                nc.scalar.sqrt(t0[:], t0[:])
                nc.vector.reciprocal(t0[:], t0[:])
                nc.gpsimd.tensor_tensor(t1[:], gc, sc, ALU.mult)
                nc.gpsimd.tensor_tensor(t1[:], t1[:], t0[:], ALU.mult)
                nc.gpsimd.tensor_copy(ab[:, oc, 0:1], t1[:])
                nc.gpsimd.tensor_tensor(t1[:], gm[:], t1[:], ALU.mult)
                nc.gpsimd.tensor_sub(ab[:, oc, 1:2], bc, t1[:])

            def allreduce(payload, width, tag, dma_in=None):
                """payload -> DRAM -> AllReduce (Pool) -> SBUF.  dma_in picks
                the queue for the payload staging DMA (default Pool); the
                result DMA is always on Pool, right behind the collective."""
                dma_in = dma_in or nc.gpsimd
                ci = dr.tile([128, width], F32, name=f"ci_{tag}")
                co = dr.tile([128, width], F32, name=f"co_{tag}")
                dma_in.dma_start(ci[:], payload)
                if no_ar:
                    nc.gpsimd.dma_start(co[:], ci[:])
                else:
                    nc.gpsimd.collective_compute(
                        "AllReduce", ALU.add,
                        replica_groups=[list(range(NCORES))],
                        ins=[ci.opt()], outs=[co.opt()])
                g = small.tile([128, width], F32, name=f"g_{tag}")
                nc.gpsimd.dma_start(g[:], co[:])
                return g

            # -------- rep-carry state for the final-oc1 pipeline ----------
            pending = []        # (front_fn, store_fn) not yet emitted
            store_lag = []      # fronts emitted, store not yet emitted

            def drain_carry(n, lag=8):
                for _ in range(n):
                    if pending:
                        fr, st = pending.pop(0)
                        fr()
                        if st is not None:
                            store_lag.append(st)
                        if len(store_lag) > lag:
                            store_lag.pop(0)()
                    elif store_lag:
                        store_lag.pop(0)()
                    else:
                        return

            # carry pacing: nothing during imgs 0-1 (ab2-oc1 of the previous
            # rep only lands ~25us into this rep's phase A -- emitting gated
            # work earlier would head-of-line block the DVE/ACT queues),
            # then 10 per image iteration.
            CARRY_PACE = [0, 1, 12, 12, 12, 12, 12, 12]

            def body(rep):
                # ================= phase A: stage0 + conv1, per-image =======
                for img in range(NIMG):
                    drain_carry(CARRY_PACE[img])
                    for j in range(2):
                        for hf in range(2):
                            xt = xst.tile([128, HHW], F32, tag="xst")
                            dma_eng = nc.sync if (j == 0 or img < 3) \
                                else nc.gpsimd
                            dma_eng.dma_start(
                                xt[:],
                                x_d[img].rearrange("c h w -> c (h w)")
                                [j * 128:(j + 1) * 128,
                                 hf * HHW:(hf + 1) * HHW])
                            # a1 = sign(x); half-image total via ACT accum
                            nc.scalar.activation(
                                interior(j, img, hf * 28, 28),
                                xt.rearrange("p (h w) -> p h w", w=W),
                                AF.Sign,
                                accum_out=sacc[:, j, img, hf:hf + 1])
                        if fast1:
                            # boundary strips for the windowed-sum trick (DVE)
                            nc.vector.tensor_reduce(
                                sacc[:, j, img, 2:3],
                                interior(j, img, 0, 1).rearrange(
                                    "p a b -> p (a b)"),
                                mybir.AxisListType.X, ALU.add)
                            nc.vector.tensor_reduce(
                                sacc[:, j, img, 3:4],
                                interior(j, img, 55, 1).rearrange(
                                    "p a b -> p (a b)"),
                                mybir.AxisListType.X, ALU.add)
                            for hf in range(2):
                                nc.vector.tensor_reduce(
                                    sacc[:, j, img, 4 + hf:5 + hf],
                                    interior(j, img, hf * 28, 28)
                                    [:, :, 0:1].rearrange(
                                        "p a b -> p (a b)"),
                                    mybir.AxisListType.X, ALU.add)
                                nc.vector.tensor_reduce(
                                    sacc[:, j, img, 6 + hf:7 + hf],
                                    interior(j, img, hf * 28, 28)
                                    [:, :, 55:56].rearrange(
                                        "p a b -> p (a b)"),
                                    mybir.AxisListType.X, ALU.add)
                    # conv1 tiles for this image, both o-chunks
                    for oc in range(2):
                        for rg in range(RG):
                            pt = pspool.tile([128, FREE], F32, tag="ps")
                            for t, (dh, dw) in enumerate(
                                    product(range(3), range(3))):
                                s = (rg * RH + dh) * WPAD + dw
                                rhs = a_buf[:, img, :, s:s + FREE]
                                nc.tensor.matmul(
                                    pt[:], wsb[:, 0, dh, dw, oc], rhs,
                                    start=(t == 0), stop=(t == 8), perf_mode=DR)
                            pv = pt.rearrange(
                                "p (r w) -> p r w", w=WPAD)[:, :, 0:56]
                            mflat = m_buf[:, oc, img, rg * VW:(rg + 1) * VW]
                            mv = mflat.rearrange("p (r w) -> p r w", w=W)
                            ti = img * RG + rg
                            if fast1:
                                # 12 DVE + 2 ACT per image (queue balance)
                                if rg == 3:
                                    nc.scalar.copy(mv, pv)
                                else:
                                    nc.vector.tensor_copy(mv, pv)
                            else:
                                nc.vector.tensor_scalar(
                                    mv, pv, 1.0, 0.0, ALU.mult, ALU.add,
                                    accum_out=acc1[:, oc, ti:ti + 1])
                                nc.vector.bn_stats(st1[:, oc, ti, :], mflat)
                drain_carry(32)  # backlog drains during the AR1 stall

                # ---- stats1: windowed-sum trick (fast1) or bn_stats path ----
                if fast1:
                    # reduce strip sums over images (Pool; gpsimd lacks
                    # free-axis reduce so chain adds)
                    pl1 = small.tile([128, 2, 8], F32, name="pl1")
                    nc.gpsimd.tensor_tensor(pl1[:], sacc[:, :, 0, :],
                                            sacc[:, :, 1, :], ALU.add)
                    for i in range(2, NIMG):
                        nc.gpsimd.tensor_tensor(pl1[:], pl1[:],
                                                sacc[:, :, i, :], ALU.add)
                    g1 = allreduce(
                        pl1.rearrange("p j s -> p (j s)"), 16, "s1")
                    gr = g1.rearrange("p (j s) -> p j s", s=8)
                    # combine halves: gv slots 0=T 1=R0 2=R55 3=C0 4=C55
                    gv = small.tile([128, 2, 5], F32, name="gv1")
                    nc.gpsimd.tensor_tensor(gv[:, :, 0:1], gr[:, :, 0:1],
                                            gr[:, :, 1:2], ALU.add)
                    nc.gpsimd.tensor_copy(gv[:, :, 1:3], gr[:, :, 2:4])
                    nc.gpsimd.tensor_tensor(gv[:, :, 3:4], gr[:, :, 4:5],
                                            gr[:, :, 5:6], ALU.add)
                    nc.gpsimd.tensor_tensor(gv[:, :, 4:5], gr[:, :, 6:7],
                                            gr[:, :, 7:8], ALU.add)
                    # S[tap] = T - (dh==0)R55 - (dh==2)R0 - (dw==0)C55
                    #            - (dw==2)C0
                    Sm = small.tile([128, 2, 9], F32, name="Sm")
                    Sm16 = small.tile([128, 2, 9], F16, name="Sm16")
                    for j in range(2):
                        rt = small.tile([128, 3], F32, name=f"rt{j}")
                        nc.gpsimd.tensor_sub(rt[:, 0:1], gv[:, j, 0:1],
                                             gv[:, j, 2:3])     # T - R55
                        nc.gpsimd.tensor_copy(rt[:, 1:2], gv[:, j, 0:1])  # T
                        nc.gpsimd.tensor_sub(rt[:, 2:3], gv[:, j, 0:1],
                                             gv[:, j, 1:2])     # T - R0
                        for t, (dh, dw) in enumerate(
                                product(range(3), range(3))):
                            dst = Sm[:, j, t:t + 1]
                            if dw == 0:
                                nc.gpsimd.tensor_sub(dst, rt[:, dh:dh + 1],
                                                     gv[:, j, 4:5])
                            elif dw == 2:
                                nc.gpsimd.tensor_sub(dst, rt[:, dh:dh + 1],
                                                     gv[:, j, 3:4])
                            else:
                                nc.gpsimd.tensor_copy(dst, rt[:, dh:dh + 1])
                    nc.gpsimd.tensor_copy(Sm16[:], Sm[:])
                    # mean1 per oc via tiny f16 matmuls: sum_n1 = w1 . S
                    for oc in range(2):
                        pmt = pmpool.tile([128, 1], F32, tag="pm")
                        for i, (j, t) in enumerate(
                                product(range(2), range(9))):
                            dh, dw = t // 3, t % 3
                            nc.tensor.matmul(
                                pmt[:], w16[:, dh, dw, oc, j],
                                Sm16[:, j, t:t + 1],
                                start=(i == 0), stop=(i == 17))
                        mu = small.tile([128, 1], F32, name=f"mu1_{oc}")
                        nc.vector.tensor_scalar_mul(mu[:], pmt[:],
                                                    -1.0 / COUNT_TOT)
                        # A1 = gamma1 ; B1 = -gamma1 * mean_n
                        nc.vector.tensor_copy(ab1[:, oc, 0:1], cf[:, oc, 1:2])
                        nc.vector.tensor_tensor(ab1[:, oc, 1:2], mu[:],
                                                cf[:, oc, 1:2], ALU.mult)
                else:
                    for oc in range(2):
                        ag1 = small.tile([128, 2], F32, name=f"ag1_{oc}")
                        pl = small.tile([128, 3], F32, name=f"plg1_{oc}")
                        nc.vector.bn_aggr(ag1[:], st1[:, oc].rearrange(
                            "p t (u s) -> p (t u) s", s=3))
                        nc.vector.tensor_copy(pl[:, 0:2], ag1[:])
                        nc.vector.tensor_tensor(pl[:, 2:3], ag1[:, 0:1],
                                                ag1[:, 0:1], ALU.mult)
                        gg = allreduce(pl[:], 3, f"s1_{oc}")
                        make_affine_old(gg, ab1, oc, 0, 1, 2)

                # ---- inner = sign(A1*n + B1), overwrites a_buf in place ----
                for img in range(NIMG):
                    for oc in range(2):
                        srcv = m_buf[:, oc, img].rearrange(
                            "p (h w) -> p h w", w=W)
                        if img == 0:
                            for hf in range(2):
                                nc.scalar.activation(
                                    interior(oc, img, hf * 28, 28),
                                    srcv[:, hf * 28:(hf + 1) * 28, :],
                                    AF.Sign, bias=ab1[:, oc, 1:2],
                                    scale=ab1[:, oc, 0:1])
                        else:
                            nc.scalar.activation(
                                interior(oc, img, 0, 56), srcv, AF.Sign,
                                bias=ab1[:, oc, 1:2], scale=ab1[:, oc, 0:1])

                # ================= phase C: conv2 + finals ==================
                bnq = []   # deferred bn_stats (keep DVE evictions at PE
                           # rate; bn gates only the stats AllReduce)

                def conv2_tile(oc, img, rg, evict, bn_lag=0):
                    pt = pspool.tile([128, FREE], F32, tag="ps")
                    for t, (dh, dw) in enumerate(
                            product(range(3), range(3))):
                        s = (rg * RH + dh) * WPAD + dw
                        rhs = a_buf[:, img, :, s:s + FREE]
                        nc.tensor.matmul(
                            pt[:], wsb[:, 1, dh, dw, oc], rhs,
                            start=(t == 0), stop=(t == 8), perf_mode=DR)
                    pv = pt.rearrange("p (r w) -> p r w", w=WPAD)[:, :, 0:56]
                    mflat = m_buf[:, oc, img, rg * VW:(rg + 1) * VW]
                    mv = mflat.rearrange("p (r w) -> p r w", w=W)
                    ti = img * RG + rg
                    evict(mv, pv, ti)
                    bnq.append((oc, ti, mflat))
                    while len(bnq) > bn_lag:
                        oc_, ti_, mf_ = bnq.pop(0)
                        nc.vector.bn_stats(st2[:, oc_, ti_, :], mf_)

                def flush_bnq():
                    while bnq:
                        oc_, ti_, mf_ = bnq.pop(0)
                        nc.vector.bn_stats(st2[:, oc_, ti_, :], mf_)

                def stats2_ar(oc):
                    """bn aggregate + AllReduce kickoff (payload DMA on the
                    ACT queue; DVE cannot issue DMAs, SP may be blocked)."""
                    ag2 = small.tile([128, 2], F32, name=f"ag2_{oc}")
                    pl2 = small.tile([128, 3], F32, name=f"pl2_{oc}")
                    nc.vector.bn_aggr(ag2[:], st2[:, oc].rearrange(
                        "p t (u s) -> p (t u) s", s=3))
                    nc.vector.tensor_copy(pl2[:, 0:2], ag2[:])
                    nc.vector.tensor_tensor(pl2[:, 2:3], ag2[:, 0:1],
                                            ag2[:, 0:1], ALU.mult)
                    return allreduce(pl2[:], 3, f"s2_{oc}", dma_in=nc.scalar)

                xtiles = {}  # (oc, img, rg) -> prefetched x ring tile

                def xload0(img, rg):
                    xr = xr0p.tile([128, VW], F32, tag="xr0")
                    nc.sync.dma_start(
                        xr[:],
                        x_d[img].rearrange("c h w -> c (h w)")
                        [0:128, rg * VW:(rg + 1) * VW])
                    xtiles[(0, img, rg)] = xr

                def final0_tile(img, rg):
                    """oc0: DVE affine_then_add in place on the x tile, ACT
                    sign in place, store on Pool."""
                    xr = xtiles.pop((0, img, rg))
                    mflat = m_buf[:, 0, img, rg * VW:(rg + 1) * VW]
                    nc.vector.affine_then_add(
                        xr[:], mflat, xr[:], ab2[:, 0, 0:1], ab2[:, 0, 1:2])
                    nc.scalar.activation(xr[:], xr[:], AF.Sign)
                    nc.gpsimd.dma_start(
                        out_d[img].rearrange("c h w -> c (h w)")
                        [0:128, rg * VW:(rg + 1) * VW],
                        xr[:])

                def push_final1(img, rg):
                    """oc1 final, carried into the next rep's phase A.
                    front: Pool x load + DVE fused affine + ACT sign;
                    store: Pool."""
                    mflat = m_buf[:, 1, img, rg * VW:(rg + 1) * VW]
                    osl = out_d[img].rearrange("c h w -> c (h w)")
                    holder = {}

                    def front():
                        xr = xr1p.tile([128, VW], F32, tag="xr1")
                        nc.gpsimd.dma_start(
                            xr[:],
                            x_d[img].rearrange("c h w -> c (h w)")
                            [128:256, rg * VW:(rg + 1) * VW])
                        nc.vector.affine_then_add(
                            xr[:], mflat, xr[:],
                            ab2[:, 1, 0:1], ab2[:, 1, 1:2])
                        nc.scalar.activation(xr[:], xr[:], AF.Sign)
                        holder["xr"] = xr

                    def store():
                        nc.sync.dma_start(
                            osl[128:256, rg * VW:(rg + 1) * VW],
                            holder["xr"])

                    pending.append((front, store))

                # conv2 oc0: evictions 2/3 DVE + 1/3 ACT (gpsimd cannot read
                # PSUM; ACT is still finishing inner); x prefetches on SP
                def evict0(mv, pv, ti):
                    if ti >= 35:
                        nc.scalar.copy(mv, pv)
                    else:
                        nc.vector.tensor_copy(mv, pv)
                for img in range(NIMG):
                    for rg in range(RG):
                        conv2_tile(0, img, rg, evict0, bn_lag=3)
                        xload0(img, rg)
                flush_bnq()
                g2_0 = stats2_ar(0)

                # conv2 oc1: evictions on ACT; affine-oc0 emitted at img3;
                # finals-oc0 interleaved 2-per-tile from img4
                def evict1(mv, pv, ti):
                    nc.scalar.copy(mv, pv)
                fq0 = [(i, r) for i in range(NIMG) for r in range(RG)]
                for img in range(NIMG):
                    for rg in range(RG):
                        conv2_tile(1, img, rg, evict1)
                        if img == 3 and rg == 0:
                            make_affine2(g2_0, ab2, 0, 3, 4, 5)
                        if img >= 4:
                            for _ in range(2):
                                if fq0:
                                    final0_tile(*fq0.pop(0))
                if sync_level >= 1:
                    tc.strict_bb_all_engine_barrier()
                for k in fq0:
                    final0_tile(*k)

                g2_1 = stats2_ar(1)
                pending.append(
                    (lambda: make_affine2(g2_1, ab2, 1, 3, 4, 5), None))
                for img in range(NIMG):
                    for rg in range(RG):
                        push_final1(img, rg)

            for _rep in range(reps):
                body(_rep)
            drain_carry(2 * NTILE + 8)   # flush the last rep's carry

    nc.compile()
    return nc


def _prep_weights(w, F8NP):
    """[O,C,3,3] fp32 -> ([128k, 3, 3, 2oc, 2j, 128m] fp8 sign, [256] fp32 scale)."""
    scale = np.mean(np.abs(w), axis=(1, 2, 3), dtype=np.float32)
    ws = np.sign(w).astype(F8NP)
    arr = ws.reshape(2, 128, 2, 128, 3, 3)       # [oc, m, j, k, dh, dw]
    arr = arr.transpose(3, 4, 5, 0, 2, 1)        # [k, dh, dw, oc, j, m]
    return np.ascontiguousarray(arr), scale


def prep_in_maps(inputs):
    """Full inputs -> (per-core in_map list, fast1 flag)."""
    x = np.ascontiguousarray(np.asarray(inputs["inputs"], dtype=np.float32))
    w1 = np.asarray(inputs["w1"], dtype=np.float32)
    w2 = np.asarray(inputs["w2"], dtype=np.float32)
    g1 = np.asarray(inputs["gamma1"], dtype=np.float32)
    b1 = np.asarray(inputs["beta1"], dtype=np.float32)
    g2 = np.asarray(inputs["gamma2"], dtype=np.float32)
    b2 = np.asarray(inputs["beta2"], dtype=np.float32)

    F8NP = ml_dtypes.float8_e4m3
    wq1, s1 = _prep_weights(w1, F8NP)
    wq2, s2 = _prep_weights(w2, F8NP)
    wq = np.ascontiguousarray(np.stack([wq1, wq2], axis=1))  # [128,2,3,3,2,2,128]
    wm = np.ascontiguousarray(wq1.astype(np.float16))        # [128,3,3,2,2,128]

    coef = np.stack([s1, g1, b1, s2, g2, b2], axis=1)  # [256, 6]
    coef = np.ascontiguousarray(
        coef.reshape(2, 128, 6).transpose(1, 0, 2).astype(np.float32))

    fast1 = bool(np.all(b1 == 0.0))
    in_maps = [
        {"x": np.ascontiguousarray(x[i * NIMG:(i + 1) * NIMG]),
         "wq": wq, "wm": wm, "cf": coef}
        for i in range(NCORES)
    ]
    return in_maps, fast1


def kernel(**inputs) -> np.ndarray:
    global LAST_RESULT
    import os
    from concourse import bass_utils

    in_maps, fast1 = prep_in_maps(inputs)
    dbg = os.environ.get("KERNEL_DEBUG", "0") == "1"
    sync_level = int(os.environ.get("KERNEL_SYNC_LEVEL", "0"))

    key = (fast1, dbg, sync_level)
    if key not in _CACHE:
        _CACHE[key] = _build(fast1, dbg, sync_level=sync_level)
    nc = _CACHE[key]

    res = bass_utils.run_bass_kernel_spmd(
        nc, in_maps, core_ids=list(range(NCORES)))
    LAST_RESULT = res
    out = np.concatenate([res.results[i]["out"] for i in range(NCORES)], axis=0)
    return out
